# revision 12
# baseline (speedup 1.0000x reference)
"""Trainium2 Bass kernel for nn_AtlasMemoryUpdate (8-core SPMD).

Computes: grads of a 2-layer MLP memory (768->512->768, gelu) under
gamma-weighted squared-error loss, then a Muon-style clamped update of
the 4 params; output = concat of updated [W1, b1, W2, b2].

Sharding: data-parallel over batch (B=16 -> 2 batches/core across 8
cores); gradients are AllReduced (3.15 MB); the tiny update is
replicated on every core; core 0's output is returned.

Layout strategy: activations token-major ([tok, feat]); transposed
operands for the feature-contraction matmuls come from large DRAM->SBUF
xbar transpose-loads of bf16 scratch copies (cheap on the HWDGE
sequencers), not per-tile SBUF->SBUF transposes.
"""

import numpy as np

import concourse.bass as bass
import concourse.mybir as mybir
import concourse.tile as tile
from concourse import bacc
from concourse.bass_utils import run_bass_kernel_spmd

# Problem shapes
B, T, D, H = 16, 2048, 768, 512
N_CORES = 8
BC = B // N_CORES          # batches per core
NTOK = BC * T               # tokens per core (4096)
P = 128
NT = NTOK // P              # token tiles per core (32)
DC = D // P                 # 6
HC = H // P                 # 4
TPB = T // P                # token tiles per batch (16)
CHUNK_TT = 2                # token tiles per phase-A chunk
CT = CHUNK_TT * P           # tokens per chunk

ETA = 0.01
BETA = 0.9
EPS = 1e-8

SZ_W1 = D * H               # 393216
SZ_B1 = H
SZ_W2 = H * D
SZ_B2 = D
OUT_SZ = SZ_W1 + SZ_B1 + SZ_W2 + SZ_B2   # 787712
OFF_B1 = SZ_W1
OFF_W2 = OFF_B1 + SZ_B1
OFF_B2 = OFF_W2 + SZ_W2

F32 = mybir.dt.float32
BF16 = mybir.dt.bfloat16
AF = mybir.ActivationFunctionType
OP = mybir.AluOpType


def build_kernel(nt=NT, use_collective=True):
    nchunk = nt // CHUNK_TT
    nc = bacc.Bacc("TRN2", target_bir_lowering=False, debug=False,
                   num_devices=N_CORES)

    keys_d = nc.declare_dram_parameter("keys", [NTOK, D], F32, isOutput=False)
    values_d = nc.declare_dram_parameter("values", [NTOK, D], F32, isOutput=False)
    gamma_d = nc.declare_dram_parameter("gamma", [T], F32, isOutput=False)
    w1_d = nc.declare_dram_parameter("W1", [D, H], F32, isOutput=False)
    b1_d = nc.declare_dram_parameter("b1", [H], F32, isOutput=False)
    w2_d = nc.declare_dram_parameter("W2", [H, D], F32, isOutput=False)
    b2_d = nc.declare_dram_parameter("b2", [D], F32, isOutput=False)
    mom_d = nc.declare_dram_parameter("momentum", [1], F32, isOutput=False)
    out_d = nc.declare_dram_parameter("out", [OUT_SZ], F32, isOutput=True)

    keys_v = keys_d[:].rearrange("(t p) d -> p t d", p=P)     # [128, 32, 768]
    vals_v = values_d[:].rearrange("(t p) d -> p t d", p=P)
    gamma_v = gamma_d[:].rearrange("(i p) -> p i", p=P)       # [128, 16]

    with tile.TileContext(nc) as tc:
        with (
            tc.tile_pool(name="const", bufs=1) as cpool,
            tc.tile_pool(name="acts", bufs=1) as apool,
            tc.tile_pool(name="dram", bufs=1, space="DRAM") as dpool,
        ):
            # ---- constants / params (bf16 compute copies) ----
            w1_bf = cpool.tile([P, DC, H], BF16)      # W1[c*128+p, h]
            w2_bf = cpool.tile([P, HC, D], BF16)      # W2[c*128+p, d]
            w2t_bf = cpool.tile([P, DC, H], BF16)     # W2[h, c*128+p]
            b1r_bf = cpool.tile([1, H], BF16)
            b2r_bf = cpool.tile([1, D], BF16)
            ones_row_b = cpool.tile([1, P], BF16)     # lhsT for bias add
            ones_col_b = cpool.tile([P, 1], BF16)     # lhsT for col-sums
            ones_row_f = cpool.tile([1, P], F32)
            gamma_sb = cpool.tile([P, TPB], F32)

            nc.gpsimd.dma_start(w1_bf[:], w1_d[:].rearrange("(c p) h -> p c h", p=P))
            nc.gpsimd.dma_start(w2_bf[:], w2_d[:].rearrange("(c p) d -> p c d", p=P))
            nc.gpsimd.dma_start(b1r_bf[:], b1_d[:].rearrange("(a h) -> a h", a=1))
            nc.gpsimd.dma_start(b2r_bf[:], b2_d[:].rearrange("(a h) -> a h", a=1))
            nc.sync.dma_start(gamma_sb[:], gamma_v)
            nc.vector.memset(ones_row_b[:], 1.0)
            nc.vector.memset(ones_col_b[:], 1.0)
            nc.vector.memset(ones_row_f[:], 1.0)
            # W2^T tiles via bf16 DMA transpose (one-time, 24 tiles)
            for dc in range(DC):
                for hc in range(HC):
                    nc.sync.dma_start(
                        w2t_bf[:, dc, hc * P:(hc + 1) * P],
                        w2_bf[:, hc, dc * P:(dc + 1) * P],
                        transpose=True,
                    )

            # ---- materialized activations (bf16, SBUF) ----
            h_all = apool.tile([P, NT, H], BF16)
            dpred_all = apool.tile([P, NT, D], BF16)
            dpre_all = apool.tile([P, NT, H], BF16)

            # ---- DRAM scratch (bf16 copies for transpose-loads) ----
            kbf_dram = [dpool.tile([CT, D], BF16, name=f"kbf_dram{i}")
                        for i in range(nchunk)]
            h_dram = [dpool.tile([CT, H], BF16, name=f"h_dram{i}")
                      for i in range(nchunk)]
            dp_dram = [dpool.tile([CT, D], BF16, name=f"dp_dram{i}")
                       for i in range(nchunk)]

            # ---- AllReduce bounce buffers (split: W1+b1 / W2+b2) ----
            SZ1 = SZ_W1 + SZ_B1
            SZ2 = SZ_W2 + SZ_B2
            cc1_in = dpool.tile([SZ1], F32)
            cc1_out = dpool.tile([SZ1], F32, addr_space="Shared")
            cc2_in = dpool.tile([SZ2], F32)
            cc2_out = dpool.tile([SZ2], F32, addr_space="Shared")

            # ======== Prep: cast keys to bf16 in DRAM scratch ==========
            with tc.tile_pool(name="prep", bufs=2) as prep:
                for ci in range(nchunk):
                    kc = prep.tile([P, CHUNK_TT, D], BF16, tag="kc")
                    sl = slice(ci * CHUNK_TT, (ci + 1) * CHUNK_TT)
                    nc.gpsimd.dma_start(kc[:], keys_v[:, sl, :])
                    nc.sync.dma_start(
                        kbf_dram[ci][:].rearrange("(t p) d -> p t d", p=P), kc[:])

            # ======== Phase A: forward + data backward =================
            # Software-pipelined over chunks so the PE stream never waits
            # on the h/dpred DRAM transpose round-trips:
            #   iteration c emits  S1(c): loads+mm1+gelu, S2(c-1): mm2+dpred,
            #   S3(c-2): mm3+dpre.
            with (
                tc.tile_pool(name="pa_sb", bufs=2) as pa,
                tc.tile_pool(name="pa_ps", bufs=4, space="PSUM") as ps_a,
                tc.tile_pool(name="pa_ps2", bufs=2, space="PSUM") as ps_b,
            ):
                vals_t = {}
                dgelu_t = {}
                hT_t = {}
                dpredT_t = {}

                def stage1(ci):
                    sl = slice(ci * CHUNK_TT, (ci + 1) * CHUNK_TT)
                    vals_ch = pa.tile([P, CHUNK_TT, D], BF16, tag="vals_ch",
                                      bufs=3, name=f"vals_ch{ci}")
                    dgelu_ch = pa.tile([P, CHUNK_TT, H], BF16, tag="dgelu_ch",
                                       bufs=4, name=f"dgelu_ch{ci}")
                    keysT = pa.tile([P, DC, CT], BF16, tag="keysT", bufs=2,
                                    name=f"keysT{ci}")
                    hT = pa.tile([P, HC, CT], BF16, tag="hT", bufs=3,
                                 name=f"hT{ci}")
                    vals_t[ci] = vals_ch
                    dgelu_t[ci] = dgelu_ch
                    hT_t[ci] = hT
                    nc.gpsimd.dma_start(vals_ch[:], vals_v[:, sl, :])
                    nc.sync.dma_start(keysT[:], kbf_dram[ci][:], transpose=True)
                    for lt in range(CHUNK_TT):
                        t = ci * CHUNK_TT + lt
                        tsl = slice(lt * P, (lt + 1) * P)
                        pre_ps = ps_a.tile([P, H], F32, tag="psA",
                                           name=f"pre_ps_{t}")
                        for dc in range(DC):
                            nc.tensor.matmul(pre_ps[:], keysT[:, dc, tsl],
                                             w1_bf[:, dc, :],
                                             start=(dc == 0), stop=False)
                        nc.tensor.matmul(pre_ps[:], ones_row_b[:], b1r_bf[:],
                                         start=False, stop=True)
                        nc.scalar.activation(h_all[:, t, :], pre_ps[:], AF.Gelu)
                        nc.scalar.activation(dgelu_ch[:, lt, :], pre_ps[:],
                                             AF.Derivative_Gelu)
                    nc.sync.dma_start(
                        h_dram[ci][:].rearrange("(t p) d -> p t d", p=P),
                        h_all[:, sl, :])
                    nc.scalar.dma_start(hT[:], h_dram[ci][:], transpose=True)

                def stage2(ci):
                    sl = slice(ci * CHUNK_TT, (ci + 1) * CHUNK_TT)
                    hT = hT_t.pop(ci)
                    vals_ch = vals_t.pop(ci)
                    dpredT = pa.tile([P, DC, CT], BF16, tag="dpredT", bufs=3,
                                     name=f"dpredT{ci}")
                    dpredT_t[ci] = dpredT
                    for lt in range(CHUNK_TT):
                        t = ci * CHUNK_TT + lt
                        tsl = slice(lt * P, (lt + 1) * P)
                        pred_ps = ps_b.tile([P, D], F32, tag="psB",
                                            name=f"pred_ps_{t}")
                        for hc in range(HC):
                            nc.tensor.matmul(pred_ps[:, 0:512], hT[:, hc, tsl],
                                             w2_bf[:, hc, 0:512],
                                             start=(hc == 0), stop=False)
                            nc.tensor.matmul(pred_ps[:, 512:768], hT[:, hc, tsl],
                                             w2_bf[:, hc, 512:768],
                                             start=(hc == 0), stop=False)
                        nc.tensor.matmul(pred_ps[:, 0:512], ones_row_b[:],
                                         b2r_bf[:, 0:512], start=False, stop=True)
                        nc.tensor.matmul(pred_ps[:, 512:768], ones_row_b[:],
                                         b2r_bf[:, 512:768], start=False,
                                         stop=True)
                        nc.vector.tensor_sub(dpred_all[:, t, :], pred_ps[:],
                                             vals_ch[:, lt, :])
                        gcol = t % TPB
                        nc.vector.tensor_scalar(
                            dpred_all[:, t, :], dpred_all[:, t, :],
                            gamma_sb[:, gcol:gcol + 1], 2.0, OP.mult, OP.mult)
                    nc.scalar.dma_start(
                        dp_dram[ci][:].rearrange("(t p) d -> p t d", p=P),
                        dpred_all[:, sl, :])
                    nc.sync.dma_start(dpredT[:], dp_dram[ci][:], transpose=True)

                def stage3(ci):
                    dpredT = dpredT_t.pop(ci)
                    dgelu_ch = dgelu_t.pop(ci)
                    for lt in range(CHUNK_TT):
                        t = ci * CHUNK_TT + lt
                        tsl = slice(lt * P, (lt + 1) * P)
                        dh_ps = ps_a.tile([P, H], F32, tag="psA",
                                          name=f"dh_ps_{t}")
                        for dc in range(DC):
                            nc.tensor.matmul(dh_ps[:], dpredT[:, dc, tsl],
                                             w2t_bf[:, dc, :],
                                             start=(dc == 0), stop=(dc == DC - 1))
                        nc.vector.tensor_mul(dpre_all[:, t, :], dh_ps[:],
                                             dgelu_ch[:, lt, :])

                for c in range(nchunk + 2):
                    if c < nchunk:
                        stage1(c)
                    if 1 <= c < nchunk + 1:
                        stage2(c - 1)
                    if c >= 2:
                        stage3(c - 2)

            # ======== Phase B: dW1 = keys^T @ dpre, db1 ================
            with (
                tc.tile_pool(name="pb_sb", bufs=2) as pb,
                tc.tile_pool(name="pb_st", bufs=1) as pbst,
                tc.tile_pool(name="pb_ps", bufs=1, space="PSUM") as ps_w1,
            ):
                dw1_ps = ps_w1.tile([P, DC, H], F32)
                db1_ps = ps_w1.tile([1, H], F32)
                for ci in range(nchunk):
                    keys_ch2 = pb.tile([P, CHUNK_TT, D], BF16, tag="keys_ch2")
                    nc.sync.dma_start(
                        keys_ch2[:],
                        kbf_dram[ci][:].rearrange("(t p) d -> p t d", p=P))
                    for lt in range(CHUNK_TT):
                        t = ci * CHUNK_TT + lt
                        first = (t == 0)
                        last = (t == nt - 1)
                        for m in range(DC):
                            nc.tensor.matmul(dw1_ps[:, m, :],
                                             keys_ch2[:, lt, m * P:(m + 1) * P],
                                             dpre_all[:, t, :],
                                             start=first, stop=last)
                        nc.tensor.matmul(db1_ps[:], ones_col_b[:],
                                         dpre_all[:, t, :],
                                         start=first, stop=last)
                st1 = pbst.tile([P, DC, H], F32)
                stb1 = pbst.tile([1, H], F32)
                for m in range(DC):
                    nc.vector.tensor_copy(st1[:, m, :], dw1_ps[:, m, :])
                nc.vector.tensor_copy(stb1[:], db1_ps[:])
                nc.sync.dma_start(
                    cc1_in[0:SZ_W1].rearrange("(m p h) -> p m h", p=P, h=H),
                    st1[:])
                nc.sync.dma_start(
                    cc1_in[SZ_W1:SZ_W1 + SZ_B1].rearrange("(a h) -> a h", a=1),
                    stb1[:])
            # AR1 (dW1+db1) overlaps phase C
            if use_collective:
                nc.gpsimd.collective_compute(
                    "AllReduce", OP.add,
                    replica_groups=[list(range(N_CORES))],
                    ins=[cc1_in.opt()], outs=[cc1_out.opt()],
                )
            else:
                nc.gpsimd.dma_start(cc1_out[:], cc1_in[:])

            # ======== Phase C: dW2 = h^T @ dpred, db2 ==================
            with (
                tc.tile_pool(name="pc_st", bufs=1) as pcst,
                tc.tile_pool(name="pc_ps", bufs=2, space="PSUM") as ps_w2,
                tc.tile_pool(name="pc_ps2", bufs=1, space="PSUM") as ps_b2,
            ):
                st2 = pcst.tile([P, HC, D], F32)
                stb2 = pcst.tile([1, D], F32)
                db2a_ps = ps_b2.tile([1, 512], F32)
                db2b_ps = ps_b2.tile([1, 256], F32)
                for t in range(nt):
                    first = (t == 0)
                    last = (t == nt - 1)
                    nc.tensor.matmul(db2a_ps[:], ones_col_b[:],
                                     dpred_all[:, t, 0:512],
                                     start=first, stop=last)
                    nc.tensor.matmul(db2b_ps[:], ones_col_b[:],
                                     dpred_all[:, t, 512:768],
                                     start=first, stop=last)
                for half in range(2):
                    dw2_ps = [ps_w2.tile([P, D], F32, tag="psW2",
                                         name=f"dw2_ps_{half}_{_i}")
                              for _i in range(2)]
                    for t in range(nt):
                        first = (t == 0)
                        last = (t == nt - 1)
                        for mi in range(2):
                            m = half * 2 + mi
                            nc.tensor.matmul(dw2_ps[mi][:, 0:512],
                                             h_all[:, t, m * P:(m + 1) * P],
                                             dpred_all[:, t, 0:512],
                                             start=first, stop=last)
                            nc.tensor.matmul(dw2_ps[mi][:, 512:768],
                                             h_all[:, t, m * P:(m + 1) * P],
                                             dpred_all[:, t, 512:768],
                                             start=first, stop=last)
                    for mi in range(2):
                        m = half * 2 + mi
                        nc.vector.tensor_copy(st2[:, m, :], dw2_ps[mi][:])
                nc.vector.tensor_copy(stb2[:, 0:512], db2a_ps[:])
                nc.vector.tensor_copy(stb2[:, 512:768], db2b_ps[:])
                nc.sync.dma_start(
                    cc2_in[0:SZ_W2].rearrange("(m p d) -> p m d", p=P, d=D),
                    st2[:])
                nc.sync.dma_start(
                    cc2_in[SZ_W2:SZ_W2 + SZ_B2].rearrange("(a d) -> a d", a=1),
                    stb2[:])

            # ======== AllReduce 2 (dW2+db2) ============================
            if use_collective:
                nc.gpsimd.collective_compute(
                    "AllReduce", OP.add,
                    replica_groups=[list(range(N_CORES))],
                    ins=[cc2_in.opt()], outs=[cc2_out.opt()],
                )
            else:
                nc.gpsimd.dma_start(cc2_out[:], cc2_in[:])

            # ======== Phase D: Muon update (replicated) ================
            with (
                tc.tile_pool(name="pd_sb", bufs=1) as pd,
                tc.tile_pool(name="pd_ps", bufs=1, space="PSUM") as ps_d,
            ):
                g1 = pd.tile([P, DC, H], F32)
                gb1 = pd.tile([P, 4], F32)
                g2 = pd.tile([P, HC, D], F32)
                gb2 = pd.tile([P, 6], F32)
                p1 = pd.tile([P, DC, H], F32)
                pb1 = pd.tile([P, 4], F32)
                p2 = pd.tile([P, HC, D], F32)
                pb2 = pd.tile([P, 6], F32)
                scratch = pd.tile([P, DC * H], F32)
                mom_sb = pd.tile([1, 1], F32)
                parts = pd.tile([P, 4], F32)
                parts_b = pd.tile([P, 4], BF16)
                s_ps = ps_d.tile([1, 4], F32)
                s_sb = pd.tile([1, 4], F32)
                gn = pd.tile([1, 4], F32)
                mbuf = pd.tile([1, 5], F32)
                tmp11 = pd.tile([1, 1], F32)
                rbuf = pd.tile([1, 4], F32)
                rbuf_b = pd.tile([1, 4], BF16)
                rb_ps = ps_d.tile([P, 4], F32)
                rb = pd.tile([P, 4], F32)

                nc.sync.dma_start(
                    g1[:], cc1_out[0:SZ_W1].rearrange("(m p h) -> p m h",
                                                      p=P, h=H))
                nc.sync.dma_start(
                    gb1[:], cc1_out[SZ_W1:SZ_W1 + SZ_B1].rearrange(
                        "(p i) -> p i", p=P))
                nc.sync.dma_start(
                    g2[:], cc2_out[0:SZ_W2].rearrange("(m p d) -> p m d",
                                                      p=P, d=D))
                nc.sync.dma_start(
                    gb2[:], cc2_out[SZ_W2:SZ_W2 + SZ_B2].rearrange(
                        "(p i) -> p i", p=P))
                nc.scalar.dma_start(
                    p1[:], w1_d[:].rearrange("(m p) h -> p m h", p=P))
                nc.scalar.dma_start(
                    pb1[:], b1_d[:].rearrange("(p i) -> p i", p=P))
                nc.scalar.dma_start(
                    p2[:], w2_d[:].rearrange("(m p) d -> p m d", p=P))
                nc.scalar.dma_start(
                    pb2[:], b2_d[:].rearrange("(p i) -> p i", p=P))
                nc.sync.dma_start(mom_sb[:], mom_d[:].rearrange("(a b) -> a b", a=1))

                glist = [
                    (g1.rearrange("p m h -> p (m h)"), DC * H),
                    (gb1[:], 4),
                    (g2.rearrange("p m d -> p (m d)"), HC * D),
                    (gb2[:], 6),
                ]
                # sum of squares per param -> parts[:, i]; cross-partition
                # reduce via bf16 matmul with a ones column (bf16 rounding
                # of gnorm is ~0.4%, far below what the update needs)
                for i, (gap, w) in enumerate(glist):
                    nc.scalar.activation(scratch[:, 0:w], gap, AF.Square,
                                         accum_out=parts[:, i:i + 1])
                nc.vector.tensor_copy(parts_b[:], parts[:])
                for i in range(4):
                    nc.tensor.matmul(s_ps[:, i:i + 1], parts_b[:, i:i + 1],
                                     ones_col_b[:], start=True, stop=True)
                nc.vector.tensor_copy(s_sb[:], s_ps[:])
                nc.scalar.activation(gn[:], s_sb[:], AF.Sqrt)

                # momentum chain: m_i = BETA*m_{i-1} + (1-BETA)*gnorm_i
                nc.vector.tensor_copy(mbuf[:, 0:1], mom_sb[:])
                for i in range(4):
                    nc.vector.tensor_scalar(
                        tmp11[:], gn[:, i:i + 1], 1.0 - BETA, None, OP.mult)
                    nc.vector.scalar_tensor_tensor(
                        mbuf[:, i + 1:i + 2], mbuf[:, i:i + 1], BETA, tmp11[:],
                        OP.mult, OP.add)
                    # r_i = ETA / (m_i + EPS)
                    nc.vector.tensor_scalar(
                        tmp11[:], mbuf[:, i + 1:i + 2], EPS, None, OP.add)
                    nc.vector.reciprocal(tmp11[:], tmp11[:])
                    nc.vector.tensor_scalar(
                        rbuf[:, i:i + 1], tmp11[:], -ETA, None, OP.mult)
                # broadcast r to all partitions (bf16 matmul)
                nc.vector.tensor_copy(rbuf_b[:], rbuf[:])
                nc.tensor.matmul(rb_ps[:], ones_row_b[:], rbuf_b[:],
                                 start=True, stop=True)
                nc.vector.tensor_copy(rb[:], rb_ps[:])

                # upd = clip(r * clip(g, +-0.1), +-0.01); out = p - upd
                outviews = [
                    (g1.rearrange("p m h -> p (m h)"),
                     p1.rearrange("p m h -> p (m h)"), p1[:],
                     out_d[0:SZ_W1].rearrange("(m p h) -> p m h", p=P, h=H)),
                    (gb1[:], pb1[:], pb1[:],
                     out_d[OFF_B1:OFF_B1 + SZ_B1].rearrange("(p i) -> p i", p=P)),
                    (g2.rearrange("p m d -> p (m d)"),
                     p2.rearrange("p m d -> p (m d)"), p2[:],
                     out_d[OFF_W2:OFF_W2 + SZ_W2].rearrange("(m p d) -> p m d",
                                                            p=P, d=D)),
                    (gb2[:], pb2[:], pb2[:],
                     out_d[OFF_B2:OFF_B2 + SZ_B2].rearrange("(p i) -> p i", p=P)),
                ]
                # upd_neg = clip(r_neg*clip(g,+-.1), +-.01); out = p + upd_neg
                for i, (gap, pap, pout, ov) in enumerate(outviews):
                    nc.vector.tensor_scalar(gap, gap, 0.1, -0.1, OP.min, OP.max)
                    nc.vector.tensor_scalar(gap, gap, rb[:, i:i + 1], -0.01,
                                            OP.mult, OP.max)
                    nc.vector.scalar_tensor_tensor(pap, gap, 0.01, pap,
                                                   OP.min, OP.add)
                    eng = nc.sync if i % 2 == 0 else nc.scalar
                    eng.dma_start(ov, pout)

    nc.compile()
    return nc


_NC_CACHE = None


def _get_nc():
    global _NC_CACHE
    if _NC_CACHE is None:
        _NC_CACHE = build_kernel()
    return _NC_CACHE


def make_in_maps(inputs):
    keys = np.ascontiguousarray(np.asarray(inputs["keys"], dtype=np.float32))
    values = np.ascontiguousarray(np.asarray(inputs["values"], dtype=np.float32))
    gamma = np.asarray(inputs["gamma"], dtype=np.float32)
    W1 = np.asarray(inputs["W1"], dtype=np.float32)
    b1 = np.asarray(inputs["b1"], dtype=np.float32)
    W2 = np.asarray(inputs["W2"], dtype=np.float32)
    b2 = np.asarray(inputs["b2"], dtype=np.float32)
    momentum = np.asarray(inputs["momentum"], dtype=np.float32)
    in_maps = []
    for c in range(N_CORES):
        ks = keys[c * BC:(c + 1) * BC].reshape(NTOK, D)
        vs = values[c * BC:(c + 1) * BC].reshape(NTOK, D)
        in_maps.append({
            "keys": np.ascontiguousarray(ks),
            "values": np.ascontiguousarray(vs),
            "gamma": gamma, "W1": W1, "b1": b1, "W2": W2, "b2": b2,
            "momentum": momentum,
        })
    return in_maps


def kernel(**inputs):
    nc = _get_nc()
    in_maps = make_in_maps(inputs)
    res = run_bass_kernel_spmd(nc, in_maps, list(range(N_CORES)))
    return res.results[0]["out"]


if __name__ == "__main__":
    rng = np.random.default_rng(0)
    inputs = {
        "keys": rng.standard_normal((B, T, D), dtype=np.float32),
        "values": rng.standard_normal((B, T, D), dtype=np.float32),
        "gamma": rng.random(T, dtype=np.float32),
        "W1": (rng.standard_normal((D, H)) / np.sqrt(D)).astype(np.float32),
        "b1": np.zeros(H, np.float32),
        "W2": (rng.standard_normal((H, D)) / np.sqrt(H)).astype(np.float32),
        "b2": np.zeros(D, np.float32),
        "momentum": np.zeros(1, np.float32),
    }
    out = kernel(**inputs)
    print("out", out.shape, out.dtype, out[:5])


# revision 14
# speedup vs baseline: 1.0303x; 1.0303x over previous
"""Trainium2 Bass kernel for nn_AtlasMemoryUpdate (8-core SPMD).

Computes: grads of a 2-layer MLP memory (768->512->768, gelu) under
gamma-weighted squared-error loss, then a Muon-style clamped update of
the 4 params; output = concat of updated [W1, b1, W2, b2].

Sharding: data-parallel over batch (B=16 -> 2 batches/core across 8
cores); gradients are AllReduced (3.15 MB); the tiny update is
replicated on every core; core 0's output is returned.

Layout strategy: activations token-major ([tok, feat]); transposed
operands for the feature-contraction matmuls come from large DRAM->SBUF
xbar transpose-loads of bf16 scratch copies (cheap on the HWDGE
sequencers), not per-tile SBUF->SBUF transposes.
"""

import numpy as np

import concourse.bass as bass
import concourse.mybir as mybir
import concourse.tile as tile
from concourse import bacc
from concourse.bass_utils import run_bass_kernel_spmd

# Problem shapes
B, T, D, H = 16, 2048, 768, 512
N_CORES = 8
BC = B // N_CORES          # batches per core
NTOK = BC * T               # tokens per core (4096)
P = 128
NT = NTOK // P              # token tiles per core (32)
DC = D // P                 # 6
HC = H // P                 # 4
TPB = T // P                # token tiles per batch (16)
CHUNK_TT = 2                # token tiles per phase-A chunk
CT = CHUNK_TT * P           # tokens per chunk

ETA = 0.01
BETA = 0.9
EPS = 1e-8

SZ_W1 = D * H               # 393216
SZ_B1 = H
SZ_W2 = H * D
SZ_B2 = D
OUT_SZ = SZ_W1 + SZ_B1 + SZ_W2 + SZ_B2   # 787712
OFF_B1 = SZ_W1
OFF_W2 = OFF_B1 + SZ_B1
OFF_B2 = OFF_W2 + SZ_W2

F32 = mybir.dt.float32
BF16 = mybir.dt.bfloat16
AF = mybir.ActivationFunctionType
OP = mybir.AluOpType


def build_kernel(nt=NT, use_collective=True):
    nchunk = nt // CHUNK_TT
    nc = bacc.Bacc("TRN2", target_bir_lowering=False, debug=False,
                   num_devices=N_CORES)

    keys_d = nc.declare_dram_parameter("keys", [NTOK, D], F32, isOutput=False)
    values_d = nc.declare_dram_parameter("values", [NTOK, D], F32, isOutput=False)
    gamma_d = nc.declare_dram_parameter("gamma", [T], F32, isOutput=False)
    w1_d = nc.declare_dram_parameter("W1", [D, H], F32, isOutput=False)
    b1_d = nc.declare_dram_parameter("b1", [H], F32, isOutput=False)
    w2_d = nc.declare_dram_parameter("W2", [H, D], F32, isOutput=False)
    b2_d = nc.declare_dram_parameter("b2", [D], F32, isOutput=False)
    mom_d = nc.declare_dram_parameter("momentum", [1], F32, isOutput=False)
    out_d = nc.declare_dram_parameter("out", [OUT_SZ], F32, isOutput=True)

    keys_v = keys_d[:].rearrange("(t p) d -> p t d", p=P)     # [128, 32, 768]
    vals_v = values_d[:].rearrange("(t p) d -> p t d", p=P)
    gamma_v = gamma_d[:].rearrange("(i p) -> p i", p=P)       # [128, 16]

    with tile.TileContext(nc) as tc:
        with (
            tc.tile_pool(name="const", bufs=1) as cpool,
            tc.tile_pool(name="acts", bufs=1) as apool,
            tc.tile_pool(name="dram", bufs=1, space="DRAM") as dpool,
        ):
            # ---- constants / params (bf16 compute copies) ----
            w1_bf = cpool.tile([P, DC, H], BF16)      # W1[c*128+p, h]
            w2_bf = cpool.tile([P, HC, D], BF16)      # W2[c*128+p, d]
            w2t_bf = cpool.tile([P, DC, H], BF16)     # W2[h, c*128+p]
            b1r_bf = cpool.tile([1, H], BF16)
            b2r_bf = cpool.tile([1, D], BF16)
            ones_row_b = cpool.tile([1, P], BF16)     # lhsT for bias add
            ones_col_b = cpool.tile([P, 1], BF16)     # lhsT for col-sums
            ones_row_f = cpool.tile([1, P], F32)
            gamma_sb = cpool.tile([P, TPB], F32)

            nc.gpsimd.dma_start(w1_bf[:], w1_d[:].rearrange("(c p) h -> p c h", p=P))
            nc.gpsimd.dma_start(w2_bf[:], w2_d[:].rearrange("(c p) d -> p c d", p=P))
            nc.gpsimd.dma_start(b1r_bf[:], b1_d[:].rearrange("(a h) -> a h", a=1))
            nc.gpsimd.dma_start(b2r_bf[:], b2_d[:].rearrange("(a h) -> a h", a=1))
            nc.sync.dma_start(gamma_sb[:], gamma_v)
            nc.vector.memset(ones_row_b[:], 1.0)
            nc.vector.memset(ones_col_b[:], 1.0)
            nc.vector.memset(ones_row_f[:], 1.0)
            # W2^T tiles via bf16 DMA transpose (one-time, 24 tiles)
            for dc in range(DC):
                for hc in range(HC):
                    nc.sync.dma_start(
                        w2t_bf[:, dc, hc * P:(hc + 1) * P],
                        w2_bf[:, hc, dc * P:(dc + 1) * P],
                        transpose=True,
                    )

            # ---- materialized activations (bf16, SBUF) ----
            h_all = apool.tile([P, NT, H], BF16)
            dpred_all = apool.tile([P, NT, D], BF16)
            dpre_all = apool.tile([P, NT, H], BF16)

            # ---- DRAM scratch (bf16 copies for transpose-loads) ----
            kbf_dram = [dpool.tile([CT, D], BF16, name=f"kbf_dram{i}")
                        for i in range(nchunk)]
            h_dram = [dpool.tile([CT, H], BF16, name=f"h_dram{i}")
                      for i in range(nchunk)]
            dp_dram = [dpool.tile([CT, D], BF16, name=f"dp_dram{i}")
                       for i in range(nchunk)]

            # ---- AllReduce bounce buffers (split: W1+b1 / W2+b2) ----
            SZ1 = SZ_W1 + SZ_B1
            SZ2 = SZ_W2 + SZ_B2
            cc1_in = dpool.tile([SZ1], F32)
            cc1_out = dpool.tile([SZ1], F32, addr_space="Shared")
            cc2_in = dpool.tile([SZ2], F32)
            cc2_out = dpool.tile([SZ2], F32, addr_space="Shared")


            # ======== Phase A: forward + data backward =================
            # Software-pipelined over chunks so the PE stream never waits
            # on the h/dpred DRAM transpose round-trips:
            #   iteration c emits  S1(c): loads+mm1+gelu, S2(c-1): mm2+dpred,
            #   S3(c-2): mm3+dpre.
            with (
                tc.tile_pool(name="pa_sb", bufs=2) as pa,
                tc.tile_pool(name="pa_ps", bufs=4, space="PSUM") as ps_a,
                tc.tile_pool(name="pa_ps2", bufs=2, space="PSUM") as ps_b,
            ):
                vals_t = {}
                dgelu_t = {}
                hT_t = {}
                dpredT_t = {}

                def stage1(ci):
                    sl = slice(ci * CHUNK_TT, (ci + 1) * CHUNK_TT)
                    vals_ch = pa.tile([P, CHUNK_TT, D], BF16, tag="vals_ch",
                                      bufs=3, name=f"vals_ch{ci}")
                    dgelu_ch = pa.tile([P, CHUNK_TT, H], BF16, tag="dgelu_ch",
                                       bufs=4, name=f"dgelu_ch{ci}")
                    keysT = pa.tile([P, DC, CT], BF16, tag="keysT", bufs=2,
                                    name=f"keysT{ci}")
                    hT = pa.tile([P, HC, CT], BF16, tag="hT", bufs=3,
                                 name=f"hT{ci}")
                    vals_t[ci] = vals_ch
                    dgelu_t[ci] = dgelu_ch
                    hT_t[ci] = hT
                    nc.gpsimd.dma_start(vals_ch[:], vals_v[:, sl, :])
                    nc.sync.dma_start(keysT[:], kbf_dram[ci][:], transpose=True)
                    for lt in range(CHUNK_TT):
                        t = ci * CHUNK_TT + lt
                        tsl = slice(lt * P, (lt + 1) * P)
                        pre_ps = ps_a.tile([P, H], F32, tag="psA",
                                           name=f"pre_ps_{t}")
                        for dc in range(DC):
                            nc.tensor.matmul(pre_ps[:], keysT[:, dc, tsl],
                                             w1_bf[:, dc, :],
                                             start=(dc == 0), stop=False)
                        nc.tensor.matmul(pre_ps[:], ones_row_b[:], b1r_bf[:],
                                         start=False, stop=True)
                        nc.scalar.activation(h_all[:, t, :], pre_ps[:], AF.Gelu)
                        nc.scalar.activation(dgelu_ch[:, lt, :], pre_ps[:],
                                             AF.Derivative_Gelu)
                    nc.scalar.dma_start(
                        h_dram[ci][:].rearrange("(t p) d -> p t d", p=P),
                        h_all[:, sl, :])
                    nc.scalar.dma_start(hT[:], h_dram[ci][:], transpose=True)

                def stage2(ci):
                    sl = slice(ci * CHUNK_TT, (ci + 1) * CHUNK_TT)
                    hT = hT_t.pop(ci)
                    vals_ch = vals_t.pop(ci)
                    dpredT = pa.tile([P, DC, CT], BF16, tag="dpredT", bufs=3,
                                     name=f"dpredT{ci}")
                    dpredT_t[ci] = dpredT
                    for lt in range(CHUNK_TT):
                        t = ci * CHUNK_TT + lt
                        tsl = slice(lt * P, (lt + 1) * P)
                        pred_ps = ps_b.tile([P, D], F32, tag="psB",
                                            name=f"pred_ps_{t}")
                        for hc in range(HC):
                            nc.tensor.matmul(pred_ps[:, 0:512], hT[:, hc, tsl],
                                             w2_bf[:, hc, 0:512],
                                             start=(hc == 0), stop=False)
                            nc.tensor.matmul(pred_ps[:, 512:768], hT[:, hc, tsl],
                                             w2_bf[:, hc, 512:768],
                                             start=(hc == 0), stop=False)
                        nc.tensor.matmul(pred_ps[:, 0:512], ones_row_b[:],
                                         b2r_bf[:, 0:512], start=False, stop=True)
                        nc.tensor.matmul(pred_ps[:, 512:768], ones_row_b[:],
                                         b2r_bf[:, 512:768], start=False,
                                         stop=True)
                        nc.vector.tensor_sub(dpred_all[:, t, :], pred_ps[:],
                                             vals_ch[:, lt, :])
                        gcol = t % TPB
                        nc.vector.tensor_scalar(
                            dpred_all[:, t, :], dpred_all[:, t, :],
                            gamma_sb[:, gcol:gcol + 1], 2.0, OP.mult, OP.mult)
                    nc.sync.dma_start(
                        dp_dram[ci][:].rearrange("(t p) d -> p t d", p=P),
                        dpred_all[:, sl, :])
                    nc.sync.dma_start(dpredT[:], dp_dram[ci][:], transpose=True)

                def stage3(ci):
                    dpredT = dpredT_t.pop(ci)
                    dgelu_ch = dgelu_t.pop(ci)
                    for lt in range(CHUNK_TT):
                        t = ci * CHUNK_TT + lt
                        tsl = slice(lt * P, (lt + 1) * P)
                        dh_ps = ps_a.tile([P, H], F32, tag="psA",
                                          name=f"dh_ps_{t}")
                        for dc in range(DC):
                            nc.tensor.matmul(dh_ps[:], dpredT[:, dc, tsl],
                                             w2t_bf[:, dc, :],
                                             start=(dc == 0), stop=(dc == DC - 1))
                        nc.vector.tensor_mul(dpre_all[:, t, :], dh_ps[:],
                                             dgelu_ch[:, lt, :])

                def prep(ci):
                    # cast keys chunk f32 -> bf16 straight in DRAM (SWDGE)
                    sl = slice(ci * CT, (ci + 1) * CT)
                    nc.gpsimd.dma_start(kbf_dram[ci][:], keys_d[sl, :])

                prep(0)
                for c in range(nchunk + 2):
                    if c + 1 < nchunk:
                        prep(c + 1)
                    if c < nchunk:
                        stage1(c)
                    if 1 <= c < nchunk + 1:
                        stage2(c - 1)
                    if c >= 2:
                        stage3(c - 2)

            # ======== Phase B: dW1 = keys^T @ dpre, db1 ================
            with (
                tc.tile_pool(name="pb_sb", bufs=2) as pb,
                tc.tile_pool(name="pb_st", bufs=1) as pbst,
                tc.tile_pool(name="pb_ps", bufs=1, space="PSUM") as ps_w1,
            ):
                dw1_ps = ps_w1.tile([P, DC, H], F32)
                db1_ps = ps_w1.tile([1, H], F32)
                for ci in range(nchunk):
                    keys_ch2 = pb.tile([P, CHUNK_TT, D], BF16, tag="keys_ch2")
                    nc.sync.dma_start(
                        keys_ch2[:],
                        kbf_dram[ci][:].rearrange("(t p) d -> p t d", p=P))
                    for lt in range(CHUNK_TT):
                        t = ci * CHUNK_TT + lt
                        first = (t == 0)
                        last = (t == nt - 1)
                        for m in range(DC):
                            nc.tensor.matmul(dw1_ps[:, m, :],
                                             keys_ch2[:, lt, m * P:(m + 1) * P],
                                             dpre_all[:, t, :],
                                             start=first, stop=last)
                        nc.tensor.matmul(db1_ps[:], ones_col_b[:],
                                         dpre_all[:, t, :],
                                         start=first, stop=last)
                st1 = pbst.tile([P, DC, H], F32)
                stb1 = pbst.tile([1, H], F32)
                for m in range(DC):
                    nc.vector.tensor_copy(st1[:, m, :], dw1_ps[:, m, :])
                nc.vector.tensor_copy(stb1[:], db1_ps[:])
                nc.sync.dma_start(
                    cc1_in[0:SZ_W1].rearrange("(m p h) -> p m h", p=P, h=H),
                    st1[:])
                nc.sync.dma_start(
                    cc1_in[SZ_W1:SZ_W1 + SZ_B1].rearrange("(a h) -> a h", a=1),
                    stb1[:])
            # AR1 (dW1+db1) overlaps phase C
            if use_collective:
                nc.gpsimd.collective_compute(
                    "AllReduce", OP.add,
                    replica_groups=[list(range(N_CORES))],
                    ins=[cc1_in.opt()], outs=[cc1_out.opt()],
                )
            else:
                nc.gpsimd.dma_start(cc1_out[:], cc1_in[:])

            # ======== Phase C: dW2 = h^T @ dpred, db2 ==================
            with (
                tc.tile_pool(name="pc_st", bufs=1) as pcst,
                tc.tile_pool(name="pc_ps", bufs=2, space="PSUM") as ps_w2,
                tc.tile_pool(name="pc_ps2", bufs=1, space="PSUM") as ps_b2,
            ):
                st2 = pcst.tile([P, HC, D], F32)
                stb2 = pcst.tile([1, D], F32)
                db2a_ps = ps_b2.tile([1, 512], F32)
                db2b_ps = ps_b2.tile([1, 256], F32)
                for t in range(nt):
                    first = (t == 0)
                    last = (t == nt - 1)
                    nc.tensor.matmul(db2a_ps[:], ones_col_b[:],
                                     dpred_all[:, t, 0:512],
                                     start=first, stop=last)
                    nc.tensor.matmul(db2b_ps[:], ones_col_b[:],
                                     dpred_all[:, t, 512:768],
                                     start=first, stop=last)
                for half in range(2):
                    dw2_ps = [ps_w2.tile([P, D], F32, tag="psW2",
                                         name=f"dw2_ps_{half}_{_i}")
                              for _i in range(2)]
                    for t in range(nt):
                        first = (t == 0)
                        last = (t == nt - 1)
                        for mi in range(2):
                            m = half * 2 + mi
                            nc.tensor.matmul(dw2_ps[mi][:, 0:512],
                                             h_all[:, t, m * P:(m + 1) * P],
                                             dpred_all[:, t, 0:512],
                                             start=first, stop=last)
                            nc.tensor.matmul(dw2_ps[mi][:, 512:768],
                                             h_all[:, t, m * P:(m + 1) * P],
                                             dpred_all[:, t, 512:768],
                                             start=first, stop=last)
                    for mi in range(2):
                        m = half * 2 + mi
                        nc.vector.tensor_copy(st2[:, m, :], dw2_ps[mi][:])
                nc.vector.tensor_copy(stb2[:, 0:512], db2a_ps[:])
                nc.vector.tensor_copy(stb2[:, 512:768], db2b_ps[:])
                nc.sync.dma_start(
                    cc2_in[0:SZ_W2].rearrange("(m p d) -> p m d", p=P, d=D),
                    st2[:])
                nc.sync.dma_start(
                    cc2_in[SZ_W2:SZ_W2 + SZ_B2].rearrange("(a d) -> a d", a=1),
                    stb2[:])

            # ======== AllReduce 2 (dW2+db2) ============================
            if use_collective:
                nc.gpsimd.collective_compute(
                    "AllReduce", OP.add,
                    replica_groups=[list(range(N_CORES))],
                    ins=[cc2_in.opt()], outs=[cc2_out.opt()],
                )
            else:
                nc.gpsimd.dma_start(cc2_out[:], cc2_in[:])

            # ======== Phase D: Muon update (replicated) ================
            # Split into two groups: group 0 (W1, b1) only needs AR1 and
            # runs while AR2 is still in flight; group 1 (W2, b2) follows.
            with (
                tc.tile_pool(name="pd_sb", bufs=1) as pd,
                tc.tile_pool(name="pd_ps", bufs=1, space="PSUM") as ps_d,
            ):
                g1 = pd.tile([P, DC, H], F32)
                gb1 = pd.tile([P, 4], F32)
                g2 = pd.tile([P, HC, D], F32)
                gb2 = pd.tile([P, 6], F32)
                p1 = pd.tile([P, DC, H], F32)
                pb1 = pd.tile([P, 4], F32)
                p2 = pd.tile([P, HC, D], F32)
                pb2 = pd.tile([P, 6], F32)
                scratch = pd.tile([P, DC * H], F32)
                mom_sb = pd.tile([1, 1], F32)
                parts = pd.tile([P, 4], F32)
                parts_b = pd.tile([P, 4], BF16)
                s_ps = ps_d.tile([1, 4], F32)
                s_sb = pd.tile([1, 4], F32)
                gn = pd.tile([1, 4], F32)
                mbuf = pd.tile([1, 5], F32)
                tmp11 = pd.tile([1, 1], F32)
                rbuf = pd.tile([1, 4], F32)
                rbuf_b = pd.tile([1, 4], BF16)
                rb_ps = ps_d.tile([P, 4], F32)
                rb = pd.tile([P, 4], F32)

                # param loads (independent of the ARs -> scheduled early)
                nc.scalar.dma_start(
                    p1[:], w1_d[:].rearrange("(m p) h -> p m h", p=P))
                nc.scalar.dma_start(
                    pb1[:], b1_d[:].rearrange("(p i) -> p i", p=P))
                nc.scalar.dma_start(
                    p2[:], w2_d[:].rearrange("(m p) d -> p m d", p=P))
                nc.scalar.dma_start(
                    pb2[:], b2_d[:].rearrange("(p i) -> p i", p=P))
                nc.sync.dma_start(mom_sb[:], mom_d[:].rearrange("(a b) -> a b", a=1))
                nc.vector.tensor_copy(mbuf[:, 0:1], mom_sb[:])

                nc.sync.dma_start(
                    g1[:], cc1_out[0:SZ_W1].rearrange("(m p h) -> p m h",
                                                      p=P, h=H))
                nc.sync.dma_start(
                    gb1[:], cc1_out[SZ_W1:SZ_W1 + SZ_B1].rearrange(
                        "(p i) -> p i", p=P))
                nc.sync.dma_start(
                    g2[:], cc2_out[0:SZ_W2].rearrange("(m p d) -> p m d",
                                                      p=P, d=D))
                nc.sync.dma_start(
                    gb2[:], cc2_out[SZ_W2:SZ_W2 + SZ_B2].rearrange(
                        "(p i) -> p i", p=P))

                groups = [
                    [(g1.rearrange("p m h -> p (m h)"), DC * H,
                      p1.rearrange("p m h -> p (m h)"), p1[:],
                      out_d[0:SZ_W1].rearrange("(m p h) -> p m h", p=P, h=H)),
                     (gb1[:], 4, pb1[:], pb1[:],
                      out_d[OFF_B1:OFF_B1 + SZ_B1].rearrange("(p i) -> p i",
                                                             p=P))],
                    [(g2.rearrange("p m d -> p (m d)"), HC * D,
                      p2.rearrange("p m d -> p (m d)"), p2[:],
                      out_d[OFF_W2:OFF_W2 + SZ_W2].rearrange("(m p d) -> p m d",
                                                             p=P, d=D)),
                     (gb2[:], 6, pb2[:], pb2[:],
                      out_d[OFF_B2:OFF_B2 + SZ_B2].rearrange("(p i) -> p i",
                                                             p=P))],
                ]
                for gi, group in enumerate(groups):
                    for k, (gap, w, pap, pout, ov) in enumerate(group):
                        i = gi * 2 + k
                        # sumsq -> parts[:, i] (ACT square + row-accum),
                        # then cross-partition reduce via bf16 matmul
                        nc.scalar.activation(scratch[:, 0:w], gap, AF.Square,
                                             accum_out=parts[:, i:i + 1])
                        nc.vector.tensor_copy(parts_b[:, i:i + 1],
                                              parts[:, i:i + 1])
                        nc.tensor.matmul(s_ps[:, i:i + 1], parts_b[:, i:i + 1],
                                         ones_col_b[:], start=True, stop=True)
                        nc.vector.tensor_copy(s_sb[:, i:i + 1], s_ps[:, i:i + 1])
                        nc.scalar.activation(gn[:, i:i + 1], s_sb[:, i:i + 1],
                                             AF.Sqrt)
                        # momentum chain step + r_i = -ETA/(m_i+EPS)
                        nc.vector.tensor_scalar(
                            tmp11[:], gn[:, i:i + 1], 1.0 - BETA, None, OP.mult)
                        nc.vector.scalar_tensor_tensor(
                            mbuf[:, i + 1:i + 2], mbuf[:, i:i + 1], BETA,
                            tmp11[:], OP.mult, OP.add)
                        nc.vector.tensor_scalar(
                            tmp11[:], mbuf[:, i + 1:i + 2], EPS, None, OP.add)
                        nc.vector.reciprocal(tmp11[:], tmp11[:])
                        nc.vector.tensor_scalar(
                            rbuf[:, i:i + 1], tmp11[:], -ETA, None, OP.mult)
                    # broadcast r for this group's two params
                    sl2 = slice(gi * 2, gi * 2 + 2)
                    nc.vector.tensor_copy(rbuf_b[:, sl2], rbuf[:, sl2])
                    nc.tensor.matmul(rb_ps[:, sl2], ones_row_b[:],
                                     rbuf_b[:, sl2], start=True, stop=True)
                    nc.vector.tensor_copy(rb[:, sl2], rb_ps[:, sl2])
                    # upd_neg = clip(r_neg*clip(g,+-.1), +-.01); out = p+upd_neg
                    for k, (gap, w, pap, pout, ov) in enumerate(group):
                        i = gi * 2 + k
                        nc.vector.tensor_scalar(gap, gap, 0.1, -0.1,
                                                OP.min, OP.max)
                        nc.vector.tensor_scalar(gap, gap, rb[:, i:i + 1],
                                                -0.01, OP.mult, OP.max)
                        nc.vector.scalar_tensor_tensor(pap, gap, 0.01, pap,
                                                       OP.min, OP.add)
                        eng = nc.sync if i % 2 == 0 else nc.scalar
                        eng.dma_start(ov, pout)

    nc.compile()
    return nc


_NC_CACHE = None


def _get_nc():
    global _NC_CACHE
    if _NC_CACHE is None:
        _NC_CACHE = build_kernel()
    return _NC_CACHE


def make_in_maps(inputs):
    keys = np.ascontiguousarray(np.asarray(inputs["keys"], dtype=np.float32))
    values = np.ascontiguousarray(np.asarray(inputs["values"], dtype=np.float32))
    gamma = np.asarray(inputs["gamma"], dtype=np.float32)
    W1 = np.asarray(inputs["W1"], dtype=np.float32)
    b1 = np.asarray(inputs["b1"], dtype=np.float32)
    W2 = np.asarray(inputs["W2"], dtype=np.float32)
    b2 = np.asarray(inputs["b2"], dtype=np.float32)
    momentum = np.asarray(inputs["momentum"], dtype=np.float32)
    in_maps = []
    for c in range(N_CORES):
        ks = keys[c * BC:(c + 1) * BC].reshape(NTOK, D)
        vs = values[c * BC:(c + 1) * BC].reshape(NTOK, D)
        in_maps.append({
            "keys": np.ascontiguousarray(ks),
            "values": np.ascontiguousarray(vs),
            "gamma": gamma, "W1": W1, "b1": b1, "W2": W2, "b2": b2,
            "momentum": momentum,
        })
    return in_maps


def kernel(**inputs):
    nc = _get_nc()
    in_maps = make_in_maps(inputs)
    res = run_bass_kernel_spmd(nc, in_maps, list(range(N_CORES)))
    return res.results[0]["out"]


if __name__ == "__main__":
    rng = np.random.default_rng(0)
    inputs = {
        "keys": rng.standard_normal((B, T, D), dtype=np.float32),
        "values": rng.standard_normal((B, T, D), dtype=np.float32),
        "gamma": rng.random(T, dtype=np.float32),
        "W1": (rng.standard_normal((D, H)) / np.sqrt(D)).astype(np.float32),
        "b1": np.zeros(H, np.float32),
        "W2": (rng.standard_normal((H, D)) / np.sqrt(H)).astype(np.float32),
        "b2": np.zeros(D, np.float32),
        "momentum": np.zeros(1, np.float32),
    }
    out = kernel(**inputs)
    print("out", out.shape, out.dtype, out[:5])


# revision 15
# speedup vs baseline: 1.1896x; 1.1547x over previous
"""Trainium2 Bass kernel for nn_AtlasMemoryUpdate (8-core SPMD).

Computes: grads of a 2-layer MLP memory (768->512->768, gelu) under
gamma-weighted squared-error loss, then a Muon-style clamped update of
the 4 params; output = concat of updated [W1, b1, W2, b2].

Sharding: data-parallel over batch (B=16 -> 2 batches/core across 8
cores); gradients are AllReduced (3.15 MB); the tiny update is
replicated on every core; core 0's output is returned.

Layout strategy: activations token-major ([tok, feat]); transposed
operands for the feature-contraction matmuls come from large DRAM->SBUF
xbar transpose-loads of bf16 scratch copies (cheap on the HWDGE
sequencers), not per-tile SBUF->SBUF transposes.
"""

import numpy as np

import concourse.bass as bass
import concourse.mybir as mybir
import concourse.tile as tile
from concourse import bacc
from concourse.bass_utils import run_bass_kernel_spmd

# Problem shapes
B, T, D, H = 16, 2048, 768, 512
N_CORES = 8
BC = B // N_CORES          # batches per core
NTOK = BC * T               # tokens per core (4096)
P = 128
NT = NTOK // P              # token tiles per core (32)
DC = D // P                 # 6
HC = H // P                 # 4
TPB = T // P                # token tiles per batch (16)
CHUNK_TT = 2                # token tiles per phase-A chunk
CT = CHUNK_TT * P           # tokens per chunk

ETA = 0.01
BETA = 0.9
EPS = 1e-8

SZ_W1 = D * H               # 393216
SZ_B1 = H
SZ_W2 = H * D
SZ_B2 = D
OUT_SZ = SZ_W1 + SZ_B1 + SZ_W2 + SZ_B2   # 787712
OFF_B1 = SZ_W1
OFF_W2 = OFF_B1 + SZ_B1
OFF_B2 = OFF_W2 + SZ_W2

F32 = mybir.dt.float32
BF16 = mybir.dt.bfloat16
AF = mybir.ActivationFunctionType
OP = mybir.AluOpType


def build_kernel(nt=NT, use_collective=True):
    nchunk = nt // CHUNK_TT
    nc = bacc.Bacc("TRN2", target_bir_lowering=False, debug=False,
                   num_devices=N_CORES)

    keys_d = nc.declare_dram_parameter("keys", [NTOK, D], F32, isOutput=False)
    values_d = nc.declare_dram_parameter("values", [NTOK, D], F32, isOutput=False)
    gamma_d = nc.declare_dram_parameter("gamma", [T], F32, isOutput=False)
    w1_d = nc.declare_dram_parameter("W1", [D, H], F32, isOutput=False)
    b1_d = nc.declare_dram_parameter("b1", [H], F32, isOutput=False)
    w2_d = nc.declare_dram_parameter("W2", [H, D], F32, isOutput=False)
    b2_d = nc.declare_dram_parameter("b2", [D], F32, isOutput=False)
    mom_d = nc.declare_dram_parameter("momentum", [1], F32, isOutput=False)
    out_d = nc.declare_dram_parameter("out", [OUT_SZ], F32, isOutput=True)

    keys_v = keys_d[:].rearrange("(t p) d -> p t d", p=P)     # [128, 32, 768]
    vals_v = values_d[:].rearrange("(t p) d -> p t d", p=P)
    gamma_v = gamma_d[:].rearrange("(i p) -> p i", p=P)       # [128, 16]

    with tile.TileContext(nc) as tc:
        with (
            tc.tile_pool(name="const", bufs=1) as cpool,
            tc.tile_pool(name="acts", bufs=1) as apool,
            tc.tile_pool(name="dram", bufs=1, space="DRAM") as dpool,
        ):
            # ---- constants / params (bf16 compute copies) ----
            w1_bf = cpool.tile([P, DC, H], BF16)      # W1[c*128+p, h]
            w2_bf = cpool.tile([P, HC, D], BF16)      # W2[c*128+p, d]
            w2t_bf = cpool.tile([P, DC, H], BF16)     # W2[h, c*128+p]
            b1r_bf = cpool.tile([1, H], BF16)
            b2r_bf = cpool.tile([1, D], BF16)
            ones_row_b = cpool.tile([1, P], BF16)     # lhsT for bias add
            ones_col_b = cpool.tile([P, 1], BF16)     # lhsT for col-sums
            ones_row_f = cpool.tile([1, P], F32)
            gamma_sb = cpool.tile([P, TPB], F32)

            nc.gpsimd.dma_start(w1_bf[:], w1_d[:].rearrange("(c p) h -> p c h", p=P))
            nc.gpsimd.dma_start(w2_bf[:], w2_d[:].rearrange("(c p) d -> p c d", p=P))
            nc.gpsimd.dma_start(b1r_bf[:], b1_d[:].rearrange("(a h) -> a h", a=1))
            nc.gpsimd.dma_start(b2r_bf[:], b2_d[:].rearrange("(a h) -> a h", a=1))
            nc.sync.dma_start(gamma_sb[:], gamma_v)
            nc.vector.memset(ones_row_b[:], 1.0)
            nc.vector.memset(ones_col_b[:], 1.0)
            nc.vector.memset(ones_row_f[:], 1.0)

            # ---- materialized activations (bf16, SBUF) ----
            h_all = apool.tile([P, NT, H], BF16)
            dpred_all = apool.tile([P, NT, D], BF16)
            dpre_all = apool.tile([P, NT, H], BF16)

            # ---- DRAM scratch (bf16 copies for transpose-loads) ----
            kbf_dram = [dpool.tile([CT, D], BF16, name=f"kbf_dram{i}")
                        for i in range(nchunk)]
            h_dram = [dpool.tile([CT, H], BF16, name=f"h_dram{i}")
                      for i in range(nchunk)]
            dp_dram = [dpool.tile([CT, D], BF16, name=f"dp_dram{i}")
                       for i in range(nchunk)]

            # W2^T via one DRAM->DRAM bf16 cast + one 3D transpose-load
            w2bf_dram = dpool.tile([H, D], BF16)
            nc.gpsimd.dma_start(w2bf_dram[:], w2_d[:])
            nc.sync.dma_start(w2t_bf[:], w2bf_dram[:], transpose=True)

            # ---- AllReduce bounce buffers (split: W1+b1 / W2+b2) ----
            SZ1 = SZ_W1 + SZ_B1
            SZ2 = SZ_W2 + SZ_B2
            cc1_in = dpool.tile([SZ1], F32)
            cc1_out = dpool.tile([SZ1], F32, addr_space="Shared")
            cc2_in = dpool.tile([SZ2], F32)
            cc2_out = dpool.tile([SZ2], F32, addr_space="Shared")


            # ======== Phase A: forward + data backward =================
            # Software-pipelined over chunks so the PE stream never waits
            # on the h/dpred DRAM transpose round-trips:
            #   iteration c emits  S1(c): loads+mm1+gelu, S2(c-1): mm2+dpred,
            #   S3(c-2): mm3+dpre.
            with (
                tc.tile_pool(name="pa_sb", bufs=2) as pa,
                tc.tile_pool(name="pa_ps", bufs=4, space="PSUM") as ps_a,
                tc.tile_pool(name="pa_ps2", bufs=2, space="PSUM") as ps_b,
            ):
                vals_t = {}
                dgelu_t = {}
                hT_t = {}
                dpredT_t = {}

                def stage1(ci):
                    sl = slice(ci * CHUNK_TT, (ci + 1) * CHUNK_TT)
                    vals_ch = pa.tile([P, CHUNK_TT, D], BF16, tag="vals_ch",
                                      bufs=5, name=f"vals_ch{ci}")
                    dgelu_ch = pa.tile([P, CHUNK_TT, H], BF16, tag="dgelu_ch",
                                       bufs=7, name=f"dgelu_ch{ci}")
                    keysT = pa.tile([P, DC, CT], BF16, tag="keysT", bufs=3,
                                    name=f"keysT{ci}")
                    hT = pa.tile([P, HC, CT], BF16, tag="hT", bufs=5,
                                 name=f"hT{ci}")
                    vals_t[ci] = vals_ch
                    dgelu_t[ci] = dgelu_ch
                    hT_t[ci] = hT
                    nc.gpsimd.dma_start(vals_ch[:], vals_v[:, sl, :])
                    nc.sync.dma_start(keysT[:], kbf_dram[ci][:], transpose=True)
                    for lt in range(CHUNK_TT):
                        t = ci * CHUNK_TT + lt
                        tsl = slice(lt * P, (lt + 1) * P)
                        pre_ps = ps_a.tile([P, H], F32, tag="psA",
                                           name=f"pre_ps_{t}")
                        for dc in range(DC):
                            nc.tensor.matmul(pre_ps[:], keysT[:, dc, tsl],
                                             w1_bf[:, dc, :],
                                             start=(dc == 0), stop=False)
                        nc.tensor.matmul(pre_ps[:], ones_row_b[:], b1r_bf[:],
                                         start=False, stop=True)
                        nc.scalar.activation(h_all[:, t, :], pre_ps[:], AF.Gelu)
                        nc.scalar.activation(dgelu_ch[:, lt, :], pre_ps[:],
                                             AF.Derivative_Gelu)
                    nc.scalar.dma_start(
                        h_dram[ci][:].rearrange("(t p) d -> p t d", p=P),
                        h_all[:, sl, :])
                    nc.scalar.dma_start(hT[:], h_dram[ci][:], transpose=True)

                def stage2(ci):
                    sl = slice(ci * CHUNK_TT, (ci + 1) * CHUNK_TT)
                    hT = hT_t.pop(ci)
                    vals_ch = vals_t.pop(ci)
                    dpredT = pa.tile([P, DC, CT], BF16, tag="dpredT", bufs=5,
                                     name=f"dpredT{ci}")
                    dpredT_t[ci] = dpredT
                    for lt in range(CHUNK_TT):
                        t = ci * CHUNK_TT + lt
                        tsl = slice(lt * P, (lt + 1) * P)
                        pred_ps = ps_b.tile([P, D], F32, tag="psB",
                                            name=f"pred_ps_{t}")
                        for hc in range(HC):
                            nc.tensor.matmul(pred_ps[:, 0:512], hT[:, hc, tsl],
                                             w2_bf[:, hc, 0:512],
                                             start=(hc == 0), stop=False)
                            nc.tensor.matmul(pred_ps[:, 512:768], hT[:, hc, tsl],
                                             w2_bf[:, hc, 512:768],
                                             start=(hc == 0), stop=False)
                        nc.tensor.matmul(pred_ps[:, 0:512], ones_row_b[:],
                                         b2r_bf[:, 0:512], start=False, stop=True)
                        nc.tensor.matmul(pred_ps[:, 512:768], ones_row_b[:],
                                         b2r_bf[:, 512:768], start=False,
                                         stop=True)
                        nc.vector.tensor_sub(dpred_all[:, t, :], pred_ps[:],
                                             vals_ch[:, lt, :])
                        gcol = t % TPB
                        nc.vector.tensor_scalar(
                            dpred_all[:, t, :], dpred_all[:, t, :],
                            gamma_sb[:, gcol:gcol + 1], 2.0, OP.mult, OP.mult)
                    nc.sync.dma_start(
                        dp_dram[ci][:].rearrange("(t p) d -> p t d", p=P),
                        dpred_all[:, sl, :])
                    nc.sync.dma_start(dpredT[:], dp_dram[ci][:], transpose=True)

                def stage3(ci):
                    dpredT = dpredT_t.pop(ci)
                    dgelu_ch = dgelu_t.pop(ci)
                    for lt in range(CHUNK_TT):
                        t = ci * CHUNK_TT + lt
                        tsl = slice(lt * P, (lt + 1) * P)
                        dh_ps = ps_a.tile([P, H], F32, tag="psA",
                                          name=f"dh_ps_{t}")
                        for dc in range(DC):
                            nc.tensor.matmul(dh_ps[:], dpredT[:, dc, tsl],
                                             w2t_bf[:, dc, :],
                                             start=(dc == 0), stop=(dc == DC - 1))
                        nc.vector.tensor_mul(dpre_all[:, t, :], dh_ps[:],
                                             dgelu_ch[:, lt, :])

                def prep(ci):
                    # cast keys chunk f32 -> bf16 straight in DRAM (SWDGE)
                    sl = slice(ci * CT, (ci + 1) * CT)
                    nc.gpsimd.dma_start(kbf_dram[ci][:], keys_d[sl, :])

                prep(0)
                for c in range(nchunk + 4):
                    if c + 1 < nchunk:
                        prep(c + 1)
                    if c < nchunk:
                        stage1(c)
                    if 2 <= c < nchunk + 2:
                        stage2(c - 2)
                    if c >= 4:
                        stage3(c - 4)

            # ======== Phase B: dW1 = keys^T @ dpre, db1 ================
            with (
                tc.tile_pool(name="pb_sb", bufs=2) as pb,
                tc.tile_pool(name="pb_st", bufs=1) as pbst,
                tc.tile_pool(name="pb_ps", bufs=1, space="PSUM") as ps_w1,
            ):
                dw1_ps = ps_w1.tile([P, DC, H], F32)
                db1_ps = ps_w1.tile([1, H], F32)
                for ci in range(nchunk):
                    keys_ch2 = pb.tile([P, CHUNK_TT, D], BF16, tag="keys_ch2")
                    nc.sync.dma_start(
                        keys_ch2[:],
                        kbf_dram[ci][:].rearrange("(t p) d -> p t d", p=P))
                    for lt in range(CHUNK_TT):
                        t = ci * CHUNK_TT + lt
                        first = (t == 0)
                        last = (t == nt - 1)
                        for m in range(DC):
                            nc.tensor.matmul(dw1_ps[:, m, :],
                                             keys_ch2[:, lt, m * P:(m + 1) * P],
                                             dpre_all[:, t, :],
                                             start=first, stop=last)
                        nc.tensor.matmul(db1_ps[:], ones_col_b[:],
                                         dpre_all[:, t, :],
                                         start=first, stop=last)
                st1 = pbst.tile([P, DC, H], F32)
                stb1 = pbst.tile([1, H], F32)
                for m in range(DC):
                    nc.vector.tensor_copy(st1[:, m, :], dw1_ps[:, m, :])
                nc.vector.tensor_copy(stb1[:], db1_ps[:])
                nc.sync.dma_start(
                    cc1_in[0:SZ_W1].rearrange("(m p h) -> p m h", p=P, h=H),
                    st1[:])
                nc.sync.dma_start(
                    cc1_in[SZ_W1:SZ_W1 + SZ_B1].rearrange("(a h) -> a h", a=1),
                    stb1[:])
            # AR1 (dW1+db1) overlaps phase C
            if use_collective:
                nc.gpsimd.collective_compute(
                    "AllReduce", OP.add,
                    replica_groups=[list(range(N_CORES))],
                    ins=[cc1_in.opt()], outs=[cc1_out.opt()],
                )
            else:
                nc.gpsimd.dma_start(cc1_out[:], cc1_in[:])

            # ======== Phase C: dW2 = h^T @ dpred, db2 ==================
            with (
                tc.tile_pool(name="pc_st", bufs=1) as pcst,
                tc.tile_pool(name="pc_ps", bufs=2, space="PSUM") as ps_w2,
                tc.tile_pool(name="pc_ps2", bufs=1, space="PSUM") as ps_b2,
            ):
                st2 = pcst.tile([P, HC, D], F32)
                stb2 = pcst.tile([1, D], F32)
                db2a_ps = ps_b2.tile([1, 512], F32)
                db2b_ps = ps_b2.tile([1, 256], F32)
                for t in range(nt):
                    first = (t == 0)
                    last = (t == nt - 1)
                    nc.tensor.matmul(db2a_ps[:], ones_col_b[:],
                                     dpred_all[:, t, 0:512],
                                     start=first, stop=last)
                    nc.tensor.matmul(db2b_ps[:], ones_col_b[:],
                                     dpred_all[:, t, 512:768],
                                     start=first, stop=last)
                for half in range(2):
                    dw2_ps = [ps_w2.tile([P, D], F32, tag="psW2",
                                         name=f"dw2_ps_{half}_{_i}")
                              for _i in range(2)]
                    for t in range(nt):
                        first = (t == 0)
                        last = (t == nt - 1)
                        for mi in range(2):
                            m = half * 2 + mi
                            nc.tensor.matmul(dw2_ps[mi][:, 0:512],
                                             h_all[:, t, m * P:(m + 1) * P],
                                             dpred_all[:, t, 0:512],
                                             start=first, stop=last)
                            nc.tensor.matmul(dw2_ps[mi][:, 512:768],
                                             h_all[:, t, m * P:(m + 1) * P],
                                             dpred_all[:, t, 512:768],
                                             start=first, stop=last)
                    for mi in range(2):
                        m = half * 2 + mi
                        nc.vector.tensor_copy(st2[:, m, :], dw2_ps[mi][:])
                nc.vector.tensor_copy(stb2[:, 0:512], db2a_ps[:])
                nc.vector.tensor_copy(stb2[:, 512:768], db2b_ps[:])
                nc.sync.dma_start(
                    cc2_in[0:SZ_W2].rearrange("(m p d) -> p m d", p=P, d=D),
                    st2[:])
                nc.sync.dma_start(
                    cc2_in[SZ_W2:SZ_W2 + SZ_B2].rearrange("(a d) -> a d", a=1),
                    stb2[:])

            # ======== AllReduce 2 (dW2+db2) ============================
            if use_collective:
                nc.gpsimd.collective_compute(
                    "AllReduce", OP.add,
                    replica_groups=[list(range(N_CORES))],
                    ins=[cc2_in.opt()], outs=[cc2_out.opt()],
                )
            else:
                nc.gpsimd.dma_start(cc2_out[:], cc2_in[:])

            # ======== Phase D: Muon update (replicated) ================
            # Split into two groups: group 0 (W1, b1) only needs AR1 and
            # runs while AR2 is still in flight; group 1 (W2, b2) follows.
            with (
                tc.tile_pool(name="pd_sb", bufs=1) as pd,
                tc.tile_pool(name="pd_ps", bufs=1, space="PSUM") as ps_d,
            ):
                g1 = pd.tile([P, DC, H], F32)
                gb1 = pd.tile([P, 4], F32)
                g2 = pd.tile([P, HC, D], F32)
                gb2 = pd.tile([P, 6], F32)
                p1 = pd.tile([P, DC, H], F32)
                pb1 = pd.tile([P, 4], F32)
                p2 = pd.tile([P, HC, D], F32)
                pb2 = pd.tile([P, 6], F32)
                scratch = pd.tile([P, DC * H], F32)
                mom_sb = pd.tile([1, 1], F32)
                parts = pd.tile([P, 4], F32)
                parts_b = pd.tile([P, 4], BF16)
                s_ps = ps_d.tile([1, 4], F32)
                s_sb = pd.tile([1, 4], F32)
                gn = pd.tile([1, 4], F32)
                mbuf = pd.tile([1, 5], F32)
                tmp11 = pd.tile([1, 1], F32)
                rbuf = pd.tile([1, 4], F32)
                rbuf_b = pd.tile([1, 4], BF16)
                rb_ps = ps_d.tile([P, 4], F32)
                rb = pd.tile([P, 4], F32)

                # param loads (independent of the ARs -> scheduled early)
                nc.scalar.dma_start(
                    p1[:], w1_d[:].rearrange("(m p) h -> p m h", p=P))
                nc.scalar.dma_start(
                    pb1[:], b1_d[:].rearrange("(p i) -> p i", p=P))
                nc.scalar.dma_start(
                    p2[:], w2_d[:].rearrange("(m p) d -> p m d", p=P))
                nc.scalar.dma_start(
                    pb2[:], b2_d[:].rearrange("(p i) -> p i", p=P))
                nc.sync.dma_start(mom_sb[:], mom_d[:].rearrange("(a b) -> a b", a=1))
                nc.vector.tensor_copy(mbuf[:, 0:1], mom_sb[:])

                nc.sync.dma_start(
                    g1[:], cc1_out[0:SZ_W1].rearrange("(m p h) -> p m h",
                                                      p=P, h=H))
                nc.sync.dma_start(
                    gb1[:], cc1_out[SZ_W1:SZ_W1 + SZ_B1].rearrange(
                        "(p i) -> p i", p=P))
                nc.sync.dma_start(
                    g2[:], cc2_out[0:SZ_W2].rearrange("(m p d) -> p m d",
                                                      p=P, d=D))
                nc.sync.dma_start(
                    gb2[:], cc2_out[SZ_W2:SZ_W2 + SZ_B2].rearrange(
                        "(p i) -> p i", p=P))

                groups = [
                    [(g1.rearrange("p m h -> p (m h)"), DC * H,
                      p1.rearrange("p m h -> p (m h)"), p1[:],
                      out_d[0:SZ_W1].rearrange("(m p h) -> p m h", p=P, h=H)),
                     (gb1[:], 4, pb1[:], pb1[:],
                      out_d[OFF_B1:OFF_B1 + SZ_B1].rearrange("(p i) -> p i",
                                                             p=P))],
                    [(g2.rearrange("p m d -> p (m d)"), HC * D,
                      p2.rearrange("p m d -> p (m d)"), p2[:],
                      out_d[OFF_W2:OFF_W2 + SZ_W2].rearrange("(m p d) -> p m d",
                                                             p=P, d=D)),
                     (gb2[:], 6, pb2[:], pb2[:],
                      out_d[OFF_B2:OFF_B2 + SZ_B2].rearrange("(p i) -> p i",
                                                             p=P))],
                ]
                for gi, group in enumerate(groups):
                    for k, (gap, w, pap, pout, ov) in enumerate(group):
                        i = gi * 2 + k
                        # sumsq -> parts[:, i] (ACT square + row-accum),
                        # then cross-partition reduce via bf16 matmul
                        nc.scalar.activation(scratch[:, 0:w], gap, AF.Square,
                                             accum_out=parts[:, i:i + 1])
                        nc.vector.tensor_copy(parts_b[:, i:i + 1],
                                              parts[:, i:i + 1])
                        nc.tensor.matmul(s_ps[:, i:i + 1], parts_b[:, i:i + 1],
                                         ones_col_b[:], start=True, stop=True)
                        nc.vector.tensor_copy(s_sb[:, i:i + 1], s_ps[:, i:i + 1])
                        nc.scalar.activation(gn[:, i:i + 1], s_sb[:, i:i + 1],
                                             AF.Sqrt)
                        # momentum chain step + r_i = -ETA/(m_i+EPS)
                        nc.vector.tensor_scalar(
                            tmp11[:], gn[:, i:i + 1], 1.0 - BETA, None, OP.mult)
                        nc.vector.scalar_tensor_tensor(
                            mbuf[:, i + 1:i + 2], mbuf[:, i:i + 1], BETA,
                            tmp11[:], OP.mult, OP.add)
                        nc.vector.tensor_scalar(
                            tmp11[:], mbuf[:, i + 1:i + 2], EPS, None, OP.add)
                        nc.vector.reciprocal(tmp11[:], tmp11[:])
                        nc.vector.tensor_scalar(
                            rbuf[:, i:i + 1], tmp11[:], -ETA, None, OP.mult)
                    # broadcast r for this group's two params
                    sl2 = slice(gi * 2, gi * 2 + 2)
                    nc.vector.tensor_copy(rbuf_b[:, sl2], rbuf[:, sl2])
                    nc.tensor.matmul(rb_ps[:, sl2], ones_row_b[:],
                                     rbuf_b[:, sl2], start=True, stop=True)
                    nc.vector.tensor_copy(rb[:, sl2], rb_ps[:, sl2])
                    # upd_neg = clip(r_neg*clip(g,+-.1), +-.01); out = p+upd_neg
                    for k, (gap, w, pap, pout, ov) in enumerate(group):
                        i = gi * 2 + k
                        nc.vector.tensor_scalar(gap, gap, 0.1, -0.1,
                                                OP.min, OP.max)
                        nc.vector.tensor_scalar(gap, gap, rb[:, i:i + 1],
                                                -0.01, OP.mult, OP.max)
                        nc.vector.scalar_tensor_tensor(pap, gap, 0.01, pap,
                                                       OP.min, OP.add)
                        eng = nc.sync if i % 2 == 0 else nc.scalar
                        eng.dma_start(ov, pout)

    nc.compile()
    return nc


_NC_CACHE = None


def _get_nc():
    global _NC_CACHE
    if _NC_CACHE is None:
        _NC_CACHE = build_kernel()
    return _NC_CACHE


def make_in_maps(inputs):
    keys = np.ascontiguousarray(np.asarray(inputs["keys"], dtype=np.float32))
    values = np.ascontiguousarray(np.asarray(inputs["values"], dtype=np.float32))
    gamma = np.asarray(inputs["gamma"], dtype=np.float32)
    W1 = np.asarray(inputs["W1"], dtype=np.float32)
    b1 = np.asarray(inputs["b1"], dtype=np.float32)
    W2 = np.asarray(inputs["W2"], dtype=np.float32)
    b2 = np.asarray(inputs["b2"], dtype=np.float32)
    momentum = np.asarray(inputs["momentum"], dtype=np.float32)
    in_maps = []
    for c in range(N_CORES):
        ks = keys[c * BC:(c + 1) * BC].reshape(NTOK, D)
        vs = values[c * BC:(c + 1) * BC].reshape(NTOK, D)
        in_maps.append({
            "keys": np.ascontiguousarray(ks),
            "values": np.ascontiguousarray(vs),
            "gamma": gamma, "W1": W1, "b1": b1, "W2": W2, "b2": b2,
            "momentum": momentum,
        })
    return in_maps


def kernel(**inputs):
    nc = _get_nc()
    in_maps = make_in_maps(inputs)
    res = run_bass_kernel_spmd(nc, in_maps, list(range(N_CORES)))
    return res.results[0]["out"]


if __name__ == "__main__":
    rng = np.random.default_rng(0)
    inputs = {
        "keys": rng.standard_normal((B, T, D), dtype=np.float32),
        "values": rng.standard_normal((B, T, D), dtype=np.float32),
        "gamma": rng.random(T, dtype=np.float32),
        "W1": (rng.standard_normal((D, H)) / np.sqrt(D)).astype(np.float32),
        "b1": np.zeros(H, np.float32),
        "W2": (rng.standard_normal((H, D)) / np.sqrt(H)).astype(np.float32),
        "b2": np.zeros(D, np.float32),
        "momentum": np.zeros(1, np.float32),
    }
    out = kernel(**inputs)
    print("out", out.shape, out.dtype, out[:5])
